# revision 2
# baseline (speedup 1.0000x reference)
"""Trainium2 Bass kernel for the diagonal OU-SDE sampler (nn_DiagOUSDE).

Math: y[b,0]=mu+noise[b,0]*sqrt(var0); y[b,t]=A_t*y[b,t-1]+mu(1-A_t)+sqrt(Q_t)*noise[b,t]
with A/Q per (t,d) exact OU transition coefficients.

Algorithm (per core, batch-sharded 8 ways):
  Split y = y_noise + y_det; y_det (mu term, batch-independent) is a tiny host
  recurrence added on device only when nonzero. For y_noise, normalize by the global
  cumulative decay G_t = prod_{r<=t} A_r: with u_t = noise_t*sqrtQ_t/G_t the scan is
  a plain running sum cum_t = sum_{s<=t} u_s and y_noise = G*cum. Time is chunked
  into 22 chunks of L=96; a chunk's prefix-sum is one PE matmul against a constant
  [97,97] fp32 weight: rows 0..95 lower-triangular ones, row 96 all-ones (the
  additive carry, injected as contraction row 96 of the rhs), col 96 duplicates the
  chunk's final cum so ScalarE can copy the carry from PSUM partition 96 (engine
  SBUF/PSUM accesses must start at partition 0/32/64/96 — hence L=96).
  DVE does the two elementwise passes (u = noise*S_u in place; y = G * cum-from-
  PSUM, evacuating PSUM at the same time).

Memory-bound problem: ~2.1MB in + 2.1MB out per batch row per core.
"""
import sys

for _p in ("/opt/trn_rl_repo", "/opt/pypackages"):
    if _p not in sys.path:
        sys.path.append(_p)

import numpy as np

import concourse.bacc as bacc
import concourse.mybir as mybir
from concourse.tile import TileContext
from concourse.bass_utils import run_bass_kernel_spmd

B, T, D = 64, 2048, 256
N_CORES = 8
B_S = B // N_CORES            # 8 batch rows per core
L = 96                        # time steps per chunk; contraction row 96 = carry
C = (T + L - 1) // L          # 22 chunks
TP = C * L                    # padded time length 2112

_f32 = np.float32


def _host_coeffs(ts, mu, log_kappa, log_sigma):
    """Per-(t,d) coefficient arrays in float32, mirroring the JAX reference."""
    ts = ts.astype(_f32)
    kappa = np.logaddexp(_f32(0.0), log_kappa.astype(_f32)).astype(_f32) + _f32(1e-6)
    sigma = np.logaddexp(_f32(0.0), log_sigma.astype(_f32)).astype(_f32) + _f32(1e-6)
    var0 = sigma * sigma / (_f32(2.0) * kappa)
    dt = np.maximum(ts[1:] - ts[:-1], _f32(1e-6))[:, None]            # [T-1,1]
    A = np.exp(-kappa[None, :] * dt).astype(_f32)                     # [T-1,D]
    two_k_dt = (_f32(2.0) * kappa[None, :] * dt).astype(_f32)
    small = (two_k_dt < _f32(1e-6)).astype(_f32)
    Q_exact = sigma**2 * (_f32(1.0) - np.exp(-two_k_dt)) / np.maximum(
        _f32(2.0) * kappa, _f32(1e-12))
    Q_taylor = sigma**2 * dt * (_f32(1.0) - kappa * dt + two_k_dt**2 / _f32(6.0))
    Q = (small * Q_taylor + (_f32(1.0) - small) * Q_exact).astype(_f32)

    A_full = np.concatenate([np.ones((1, D), _f32), A], axis=0)       # A_0 := 1
    sqrtQ_full = np.sqrt(
        np.concatenate([var0[None, :], Q], axis=0)).astype(_f32)      # [T,D]
    b_full = np.concatenate(
        [mu[None, :].astype(_f32), (mu[None, :] * (_f32(1.0) - A)).astype(_f32)],
        axis=0)

    logG = np.cumsum(np.log(A_full.astype(np.float64)), axis=0)
    G = np.exp(logG).astype(_f32)
    S_u = (sqrtQ_full * np.exp(-logG)).astype(_f32)                   # u = noise*S_u

    if np.any(b_full != 0):
        ydet = np.empty((T, D), _f32)
        y = b_full[0].copy()
        ydet[0] = y
        for t in range(1, T):
            y = A_full[t] * y + b_full[t]
            ydet[t] = y
    else:
        ydet = None
    return S_u, G, ydet


def _pad_tp(a):
    out = np.zeros((TP, D), _f32)
    out[:T] = a
    return out


def _tri_weight():
    # [97, 97]: W[s,t]=1{s<=t} (s,t<96); row 96 = carry (all ones);
    # col 96 duplicates col 95 => psum row 96 = chunk-final cum (the next carry)
    w = np.zeros((97, 97), _f32)
    for s in range(L):
        w[s, s:L] = _f32(1.0)
    w[L, :L] = _f32(1.0)
    w[:, L] = w[:, L - 1]
    return w


def _build_nc(with_ydet, n_iters=1):
    """Bass program for one core. noise/yout are [B_S, TP, D] in DRAM."""
    nc = bacc.Bacc("TRN2", target_bir_lowering=False, debug=False,
                   num_devices=N_CORES)
    dt32 = mybir.dt.float32
    noise = nc.dram_tensor("noise", [B_S, TP, D], dt32, kind="ExternalInput")
    s_u = nc.dram_tensor("s_u", [TP, D], dt32, kind="ExternalInput")
    g = nc.dram_tensor("g", [TP, D], dt32, kind="ExternalInput")
    tri = nc.dram_tensor("tri", [L + 1, L + 1], dt32, kind="ExternalInput")
    ydet = (nc.dram_tensor("ydet", [TP, D], dt32, kind="ExternalInput")
            if with_ydet else None)
    yout = nc.dram_tensor("yout", [B_S, TP, D], dt32, kind="ExternalOutput")

    with TileContext(nc) as tc:
        with (
            tc.tile_pool(name="coef", bufs=1) as coef,
            tc.tile_pool(name="upool", bufs=2) as upool,
            tc.tile_pool(name="ypool", bufs=2) as ypool,
            tc.tile_pool(name="psum", bufs=8, space="PSUM") as pspool,
        ):
            def body(_iv=None):
                w_t = coef.tile([L + 1, L + 1], dt32, tag="w")
                su_t = coef.tile([128, C, D], dt32, tag="su")
                g_t = coef.tile([128, C, D], dt32, tag="g")
                nc.sync.dma_start(out=w_t[:], in_=tri[:])
                nc.sync.dma_start(
                    out=su_t[0:L, :, :],
                    in_=s_u[:].rearrange("(c p) d -> p c d", p=L))
                nc.sync.dma_start(
                    out=g_t[0:L, :, :],
                    in_=g[:].rearrange("(c p) d -> p c d", p=L))
                if with_ydet:
                    yd_t = coef.tile([128, C, D], dt32, tag="yd")
                    nc.sync.dma_start(
                        out=yd_t[0:L, :, :],
                        in_=ydet[:].rearrange("(c p) d -> p c d", p=L))

                for b in range(B_S):
                    u = upool.tile([128, C, D], dt32)
                    nc.sync.dma_start(
                        out=u[0:L, :, :],
                        in_=noise[b].rearrange("(c p) d -> p c d", p=L))
                    nc.gpsimd.memset(u[L:L + 1, 0, :], 0.0)  # chunk-0 carry
                    nc.vector.tensor_mul(out=u[0:L, :, :], in0=u[0:L, :, :],
                                         in1=su_t[0:L, :, :])
                    y = ypool.tile([128, C, D], dt32)
                    for c2 in range(C // 2):
                        ps = pspool.tile([L + 1, 2 * D], dt32)
                        for h in range(2):
                            c = 2 * c2 + h
                            nc.tensor.matmul(ps[:, h * D:(h + 1) * D],
                                             w_t[:], u[0:L + 1, c, :],
                                             start=True, stop=True)
                            if c + 1 < C:
                                # next chunk's additive carry = dup'd last cum row
                                nc.scalar.copy(u[L:L + 1, c + 1, :],
                                               ps[L:L + 1, h * D:(h + 1) * D])
                        nc.vector.tensor_mul(out=y[0:L, 2 * c2:2 * c2 + 2, :],
                                             in0=g_t[0:L, 2 * c2:2 * c2 + 2, :],
                                             in1=ps[0:L, :])
                    if with_ydet:
                        nc.vector.tensor_add(out=y[0:L, :, :], in0=y[0:L, :, :],
                                             in1=yd_t[0:L, :, :])
                    nc.sync.dma_start(
                        out=yout[b].rearrange("(c p) d -> p c d", p=L),
                        in_=y[0:L, :, :])

            if n_iters == 1:
                body()
            else:
                with tc.For_i(0, n_iters, 1) as _i:
                    body(_i)
    nc.compile()
    return nc


_CACHE = {}


def _get_nc(with_ydet, n_iters=1):
    key = (with_ydet, n_iters)
    if key not in _CACHE:
        _CACHE[key] = _build_nc(with_ydet, n_iters)
    return _CACHE[key]


def _make_in_maps(ts, noise, mu, log_kappa, log_sigma):
    S_u, G, ydet = _host_coeffs(np.asarray(ts), np.asarray(mu),
                                np.asarray(log_kappa), np.asarray(log_sigma))
    su_p = _pad_tp(S_u)
    g_p = _pad_tp(G)
    yd_p = _pad_tp(ydet) if ydet is not None else None
    tri = _tri_weight()
    noise = np.ascontiguousarray(np.asarray(noise), dtype=_f32)

    in_maps = []
    for core in range(N_CORES):
        shard = noise[core * B_S:(core + 1) * B_S]        # [B_S, T, D]
        npad = np.zeros((B_S, TP, D), _f32)
        npad[:, :T] = shard
        m = {"noise": npad, "s_u": su_p, "g": g_p, "tri": tri}
        if yd_p is not None:
            m["ydet"] = yd_p
        in_maps.append(m)
    return in_maps, yd_p is not None


def kernel(ts, noise, mu, log_kappa, log_sigma):
    in_maps, with_ydet = _make_in_maps(ts, noise, mu, log_kappa, log_sigma)
    nc = _get_nc(with_ydet)
    res = run_bass_kernel_spmd(nc, in_maps, list(range(N_CORES)))
    out = np.empty((B, T, D), _f32)
    for core in range(N_CORES):
        out[core * B_S:(core + 1) * B_S] = res.results[core]["yout"][:, :T, :]
    return out
